# revision 1
# baseline (speedup 1.0000x reference)
"""CenterLoss on Trainium2 (Bass/Tile), 8-core data-parallel.

Reference semantics (see problem):
    distmat[b, c] = ||x_b||^2 + ||center_c||^2 - 2 <x_b, center_c>
    dist = distmat * onehot(labels)
    loss = sum(clip(dist, 1e-12, 1e12)) / B

Only the B "true-label" entries of distmat survive the mask; every other
entry is exactly 0 and clips to 1e-12.  So:

    loss = ( sum_b clip(||x_b - centers[labels_b]||^2, 1e-12, 1e12)
             + (B*C - B) * 1e-12 ) / B

The kernel therefore never materializes distmat: each core takes 512 batch
rows, gathers the 512 matching center rows with an indirect DMA, and
computes per-row squared distances on-chip (DVE subtract + ACT square with
row-sum accumulate).  The host sums the 4096 per-row values (f64) and adds
the deterministic clip constant.
"""

import numpy as np

from concourse import bass, bass_utils, mybir
import concourse.bacc as bacc
import concourse.tile as tile

B = 4096        # batch
D = 512         # feature dim
C = 10000       # num classes
N_CORES = 8
SHARD = B // N_CORES    # 512 rows per core
P = 128                 # SBUF partitions
NT = SHARD // P         # 4 row-tiles per core

_FP = mybir.dt.float32
_INT = mybir.dt.int32

_NC_CACHE = {}


def build_bass(enable_asserts: bool = False):
    """Build + compile the per-core Bass module (same program on all cores)."""
    nc = bacc.Bacc(
        "TRN2",
        target_bir_lowering=False,
        debug=False,
        enable_asserts=enable_asserts,
        num_devices=N_CORES,
    )
    x_d = nc.dram_tensor("x", [SHARD, D], _FP, kind="ExternalInput")
    lbl_d = nc.dram_tensor("labels", [SHARD], _INT, kind="ExternalInput")
    cen_d = nc.dram_tensor("centers", [C, D], _FP, kind="ExternalInput")
    out_d = nc.dram_tensor("out", [P, NT], _FP, kind="ExternalOutput")

    with tile.TileContext(nc) as tc:
        with (
            tc.tile_pool(name="work", bufs=NT) as work,
            tc.tile_pool(name="persist", bufs=1) as persist,
        ):
            # labels laid out so column t holds the 128 labels of row-tile t:
            # idx[p, t] = labels[t*128 + p]
            idx = persist.tile([P, NT], _INT)
            nc.sync.dma_start(out=idx[:], in_=lbl_d.ap().rearrange("(t p) -> p t", p=P))

            rowsum = persist.tile([P, NT], _FP)
            for t in range(NT):
                x_t = work.tile([P, D], _FP, tag="x")
                c_t = work.tile([P, D], _FP, tag="c")
                diff = work.tile([P, D], _FP, tag="diff")
                sq = work.tile([P, D], _FP, tag="sq")

                nc.sync.dma_start(out=x_t[:], in_=x_d.ap()[t * P : (t + 1) * P, :])
                nc.gpsimd.indirect_dma_start(
                    out=c_t[:],
                    out_offset=None,
                    in_=cen_d.ap(),
                    in_offset=bass.IndirectOffsetOnAxis(ap=idx[:, t : t + 1], axis=0),
                )
                nc.vector.tensor_tensor(
                    out=diff[:], in0=x_t[:], in1=c_t[:], op=mybir.AluOpType.subtract
                )
                # sq = diff^2 (discarded); rowsum[:, t] = sum_d diff^2
                nc.scalar.activation(
                    out=sq[:],
                    in_=diff[:],
                    func=mybir.ActivationFunctionType.Square,
                    accum_out=rowsum[:, t : t + 1],
                )

            nc.sync.dma_start(out=out_d.ap(), in_=rowsum[:])
    nc.compile()
    return nc


def _get_nc():
    if "nc" not in _NC_CACHE:
        _NC_CACHE["nc"] = build_bass()
    return _NC_CACHE["nc"]


def make_in_maps(x, labels, centers):
    x = np.ascontiguousarray(np.asarray(x, dtype=np.float32))
    labels_i32 = np.ascontiguousarray(np.asarray(labels).astype(np.int32))
    centers = np.ascontiguousarray(np.asarray(centers, dtype=np.float32))
    return [
        {
            "x": x[i * SHARD : (i + 1) * SHARD],
            "labels": labels_i32[i * SHARD : (i + 1) * SHARD],
            "centers": centers,
        }
        for i in range(N_CORES)
    ]


def finish(results):
    """Host-side unshard: per-row distances -> scalar loss (f64 accumulate)."""
    # out[p, t] = d for shard row t*128 + p  ->  transpose to (t, p) order
    d = np.concatenate([np.asarray(r["out"]).T.reshape(-1) for r in results])
    total = np.clip(d.astype(np.float64), 1e-12, 1e12).sum()
    total += (B * C - B) * 1e-12  # masked-out zeros, clipped to 1e-12 each
    return np.asarray(total / B, dtype=np.float32)


def run(x, labels, centers, trace: bool = False):
    """Run on the 8 NeuronCores; returns (loss, BassKernelResults)."""
    nc = _get_nc()
    res = bass_utils.run_bass_kernel_spmd(
        nc,
        make_in_maps(x, labels, centers),
        core_ids=list(range(N_CORES)),
        trace=trace,
    )
    return finish(res.results), res


def kernel(x, labels, centers):
    loss, _ = run(x, labels, centers)
    return loss


# revision 2
# speedup vs baseline: 1.2032x; 1.2032x over previous
"""CenterLoss on Trainium2 (Bass/Tile), 8-core data-parallel.

Reference semantics (see problem):
    distmat[b, c] = ||x_b||^2 + ||center_c||^2 - 2 <x_b, center_c>
    dist = distmat * onehot(labels)
    loss = sum(clip(dist, 1e-12, 1e12)) / B

Only the B "true-label" entries of distmat survive the mask; every other
entry is exactly 0 and clips to 1e-12.  So:

    loss = ( sum_b clip(||x_b - centers[labels_b]||^2, 1e-12, 1e12)
             + (B*C - B) * 1e-12 ) / B

The kernel therefore never materializes distmat: each core takes 512 batch
rows, gathers the 512 matching center rows with an indirect DMA, and
computes per-row squared distances on-chip (DVE subtract + ACT square with
row-sum accumulate).  The host sums the 4096 per-row values (f64) and adds
the deterministic clip constant.
"""

import numpy as np

from concourse import bass, bass_utils, mybir
import concourse.bacc as bacc
import concourse.tile as tile

B = 4096        # batch
D = 512         # feature dim
C = 10000       # num classes
N_CORES = 8
SHARD = B // N_CORES    # 512 rows per core
P = 128                 # SBUF partitions
NT = SHARD // P         # 4 row-tiles per core

_FP = mybir.dt.float32
_INT = mybir.dt.int32

_NC_CACHE = {}


def build_bass(enable_asserts: bool = False):
    """Build + compile the per-core Bass module (same program on all cores)."""
    nc = bacc.Bacc(
        "TRN2",
        target_bir_lowering=False,
        debug=False,
        enable_asserts=enable_asserts,
        num_devices=N_CORES,
    )
    x_d = nc.dram_tensor("x", [SHARD, D], _FP, kind="ExternalInput")
    lbl_d = nc.dram_tensor("labels", [SHARD], _INT, kind="ExternalInput")
    cen_d = nc.dram_tensor("centers", [C, D], _FP, kind="ExternalInput")
    out_d = nc.dram_tensor("out", [P, NT], _FP, kind="ExternalOutput")

    with tile.TileContext(nc) as tc:
        with (
            tc.tile_pool(name="work", bufs=NT) as work,
            tc.tile_pool(name="persist", bufs=1) as persist,
        ):
            # labels laid out so column t holds the 128 labels of row-tile t:
            # idx[p, t] = labels[t*128 + p]
            idx = persist.tile([P, NT], _INT)
            nc.sync.dma_start(out=idx[:], in_=lbl_d.ap().rearrange("(t p) -> p t", p=P))

            rowsum = persist.tile([P, NT], _FP)
            for t in range(NT):
                x_t = work.tile([P, D], _FP, tag="x")
                c_t = work.tile([P, D], _FP, tag="c")
                diff = work.tile([P, D], _FP, tag="diff")
                sq = work.tile([P, D], _FP, tag="sq")

                nc.sync.dma_start(out=x_t[:], in_=x_d.ap()[t * P : (t + 1) * P, :])
                nc.gpsimd.indirect_dma_start(
                    out=c_t[:],
                    out_offset=None,
                    in_=cen_d.ap(),
                    in_offset=bass.IndirectOffsetOnAxis(ap=idx[:, t : t + 1], axis=0),
                )
                nc.vector.tensor_tensor(
                    out=diff[:], in0=x_t[:], in1=c_t[:], op=mybir.AluOpType.subtract
                )
                # sq = diff^2 (discarded); rowsum[:, t] = sum_d diff^2
                nc.scalar.activation(
                    out=sq[:],
                    in_=diff[:],
                    func=mybir.ActivationFunctionType.Square,
                    accum_out=rowsum[:, t : t + 1],
                )

            nc.sync.dma_start(out=out_d.ap(), in_=rowsum[:])
    nc.compile()
    return nc


def _get_nc():
    if "nc" not in _NC_CACHE:
        _NC_CACHE["nc"] = build_bass()
    return _NC_CACHE["nc"]


def make_in_maps(x, labels, centers):
    x = np.ascontiguousarray(np.asarray(x, dtype=np.float32))
    labels_i32 = np.ascontiguousarray(np.asarray(labels).astype(np.int32))
    centers = np.ascontiguousarray(np.asarray(centers, dtype=np.float32))
    return [
        {
            "x": x[i * SHARD : (i + 1) * SHARD],
            "labels": labels_i32[i * SHARD : (i + 1) * SHARD],
            "centers": centers,
        }
        for i in range(N_CORES)
    ]


def _unshard(results):
    # out[p, t] = d for shard row t*128 + p  ->  transpose to (t, p) order
    return np.concatenate([np.asarray(r["out"]).T.reshape(-1) for r in results])


def finish(results):
    """Host-side unshard: per-row distances -> scalar loss (f64 accumulate)."""
    d = _unshard(results)
    total = np.clip(d.astype(np.float64), 1e-12, 1e12).sum()
    total += (B * C - B) * 1e-12  # masked-out zeros, clipped to 1e-12 each
    return np.asarray(total / B, dtype=np.float32)


def run(x, labels, centers, trace: bool = False):
    """Run on the 8 NeuronCores; returns (loss, BassKernelResults)."""
    nc = _get_nc()
    res = bass_utils.run_bass_kernel_spmd(
        nc,
        make_in_maps(x, labels, centers),
        core_ids=list(range(N_CORES)),
        trace=trace,
    )
    return finish(res.results), res


def kernel(x, labels, centers):
    loss, _ = run(x, labels, centers)
    return loss
